# revision 18
# baseline (speedup 1.0000x reference)
"""Trainium2 Bass kernel for nn_DiscoMLPAgent (dense_mlp).

Data-parallel over 8 NeuronCores: obs_seq sharded along B (32 batch rows
per core -> 1024 flattened rows per core), parameters replicated.

On-chip layout is feature-major (features on SBUF partitions, rows on the
free dimension) for the backbone, LSTM, and the wide output heads (y, z, q),
and rows-major for the narrow heads (logits, aux_pi).  Matmul operands are
bf16 (fp32 PSUM accumulation); elementwise state stays fp32.

tanh is computed as 2*sigmoid(2x)-1 so every activation (Sigmoid, Relu,
Identity, Copy) lives in one ACT table set ("sigmoid_and_friends") and the
~2.7us table-switch cost is paid once.  Elementwise work is spread across
ACT, DVE and GpSimd; the LSTM cell uses scalar_tensor_tensor to keep the
recurrence chain short:  c' = sf*c + (2*(si*sg) - si),  h = 2*(so*sc) - so.
"""

import numpy as np
import ml_dtypes

import concourse.bass as bass
import concourse.bacc as bacc
import concourse.tile as tile
import concourse.mybir as mybir
from concourse.bass_utils import run_bass_kernel_spmd

T, B, OBS, A, P, NB, H = 32, 256, 1024, 16, 600, 601, 128
N_CORES = 8
BC = B // N_CORES          # batch rows per core
R = T * BC                 # flattened rows per core (1024)
RB = 512                   # row block (matmul moving free dim)
NBLK = R // RB             # 2
KO = OBS // 128            # 8 k-tiles for obs
KE = 512 // 128            # 4 k-tiles for 512-wide features
YM = [128, 128, 128, 128, 88]    # y / z feature m-tile widths (600)
QM = [128, 128, 128, 128, 89]    # q feature m-tile widths (601)

F32 = mybir.dt.float32
MD = mybir.dt.bfloat16
MDnp = ml_dtypes.bfloat16
OUT_BF16 = True            # z/q written to HBM as bf16, upconverted on host
OD = MD if OUT_BF16 else F32
AF = mybir.ActivationFunctionType
ALU = mybir.AluOpType

_CACHE = {}


def _build(u_bias_nonzero: bool):
    nc = bacc.Bacc(None, target_bir_lowering=False)

    # ---- DRAM I/O (per-core). Host pre-arranges everything to [part, free...]
    xt_d = nc.dram_tensor("xt", [128, KO, R], MD, kind="ExternalInput")
    w1_d = nc.dram_tensor("w1", [128, KO, 512], MD, kind="ExternalInput")
    w2_d = nc.dram_tensor("w2", [128, KE, 512], MD, kind="ExternalInput")
    ciw_d = nc.dram_tensor("ciw", [128, KE, 128], MD, kind="ExternalInput")
    yw_d = nc.dram_tensor("yw", [128, KE, P], MD, kind="ExternalInput")
    polw_d = nc.dram_tensor("polw", [128, KE, A], MD, kind="ExternalInput")
    whh_d = nc.dram_tensor("whh", [128, 4 * H], MD, kind="ExternalInput")
    zw1_d = nc.dram_tensor("zw1", [128, H], MD, kind="ExternalInput")
    qw1_d = nc.dram_tensor("qw1", [128, H], MD, kind="ExternalInput")
    aw1_d = nc.dram_tensor("aw1", [128, H], MD, kind="ExternalInput")
    zw2_d = nc.dram_tensor("zw2", [128, P], MD, kind="ExternalInput")
    qw2_d = nc.dram_tensor("qw2", [128, NB], MD, kind="ExternalInput")
    aw2_d = nc.dram_tensor("aw2", [128, A], MD, kind="ExternalInput")
    gb_d = nc.dram_tensor("gb", [128, A, 4], F32, kind="ExternalInput")
    bb1_d = nc.dram_tensor("bb1", [128, KE], F32, kind="ExternalInput")
    bb2_d = nc.dram_tensor("bb2", [128, KE], F32, kind="ExternalInput")
    cib_d = nc.dram_tensor("cib", [128, 2], F32, kind="ExternalInput")
    ub_d = nc.dram_tensor("ub", [1, 3, 128], MD, kind="ExternalInput")

    zt_d = nc.dram_tensor("zt", [A, P, R], OD, kind="ExternalOutput")
    qt_d = nc.dram_tensor("qt", [A, NB, R], OD, kind="ExternalOutput")
    yt_d = nc.dram_tensor("yt", [P, R], F32, kind="ExternalOutput")
    lg_d = nc.dram_tensor("lg", [R, A], F32, kind="ExternalOutput")
    ax_d = nc.dram_tensor("ax", [A, R, A], F32, kind="ExternalOutput")

    # greedy cost-balanced assignment of PSUM->SBUF copies; counters are
    # primed with each engine's fixed (non-copy) load in ns.
    eng_load = {"V": 93000.0, "A": 110000.0}

    def evac(dst, src):
        """PSUM -> SBUF copy, assigned to the less-loaded of DVE/ACT."""
        n = 1
        for d in dst.shape[1:]:
            n *= d
        if eng_load["V"] <= eng_load["A"]:
            eng_load["V"] += (n + 178) / 0.96
            nc.vector.tensor_copy(dst, src)
        else:
            eng_load["A"] += (n + 352) / 1.2
            nc.scalar.activation(dst, src, AF.Copy)

    def relu(dst, src):
        """PSUM -> SBUF relu, assigned to the less-loaded of DVE/ACT."""
        n = 1
        for d in dst.shape[1:]:
            n *= d
        if eng_load["V"] <= eng_load["A"]:
            eng_load["V"] += (n + 178) / 0.96
            nc.vector.tensor_scalar_max(dst, src, 0.0)
        else:
            eng_load["A"] += (n + 352) / 1.2
            nc.scalar.activation(dst, src, AF.Relu)

    with tile.TileContext(nc) as tc:
        with (
            tc.tile_pool(name="const", bufs=1) as cp,
            tc.tile_pool(name="xtp", bufs=2) as xtp,
            tc.tile_pool(name="actp", bufs=2) as actp,
            tc.tile_pool(name="state", bufs=1) as stp,
            tc.tile_pool(name="tmp", bufs=16) as tmp,
            tc.tile_pool(name="up", bufs=3) as upool,
            tc.tile_pool(name="ostage", bufs=6) as ost,
            tc.tile_pool(name="sstage", bufs=6) as sst,
            tc.tile_pool(name="g_ps", bufs=2, space=bass.MemorySpace.PSUM) as g_ps,
            tc.tile_pool(name="misc_ps", bufs=3, space=bass.MemorySpace.PSUM) as misc_ps,
        ):
            # ---- load constants
            def cload(dram, shape, dtype, nm):
                t = cp.tile(shape, dtype, name=nm, tag=nm)
                nc.sync.dma_start(t[:], dram[:])
                return t

            w1 = cload(w1_d, [128, KO, 512], MD, "w1")
            w2 = cload(w2_d, [128, KE, 512], MD, "w2")
            ciw = cload(ciw_d, [128, KE, 128], MD, "ciw")
            yw = cload(yw_d, [128, KE, P], MD, "yw")
            polw = cload(polw_d, [128, KE, A], MD, "polw")
            whh = cload(whh_d, [128, 4 * H], MD, "whh")
            zw1 = cload(zw1_d, [128, H], MD, "zw1")
            qw1 = cload(qw1_d, [128, H], MD, "qw1")
            aw1 = cload(aw1_d, [128, H], MD, "aw1")
            zw2 = cload(zw2_d, [128, P], MD, "zw2")
            qw2 = cload(qw2_d, [128, NB], MD, "qw2")
            aw2 = cload(aw2_d, [128, A], MD, "aw2")
            gb = cload(gb_d, [128, A, 4], F32, "gb")
            bb1 = cload(bb1_d, [128, KE], F32, "bb1")
            bb2 = cload(bb2_d, [128, KE], F32, "bb2")
            cib = cload(cib_d, [128, 2], F32, "cib")
            if u_bias_nonzero:
                ub = cload(ub_d, [1, 3, 128], MD, "ub")
                ones = cp.tile([1, RB], MD, name="ones", tag="ones")
                nc.vector.memset(ones[:], 1.0)

            embs, h0s, c0s = [], [], []

            # ---- backbone + feedforward heads, per row-block
            for blk in range(NBLK):
                sl = slice(blk * RB, (blk + 1) * RB)
                xt = xtp.tile([128, KO, RB], MD, name=f"xt{blk}", tag="xt")
                nc.sync.dma_start(xt[:], xt_d[:, :, sl])

                h1 = actp.tile([128, KE, RB], MD, name=f"h1_{blk}", tag="h1")
                for mp in range(2):
                    ps = misc_ps.tile([128, 2, RB], F32, name=f"b1p{blk}_{mp}", tag="ps")
                    for j in range(2):
                        m = mp * 2 + j
                        for k in range(KO):
                            nc.tensor.matmul(ps[:, j, :], w1[:, k, m * 128:(m + 1) * 128],
                                             xt[:, k, :], start=(k == 0), stop=(k == KO - 1))
                    for j in range(2):
                        m = mp * 2 + j
                        nc.scalar.activation(h1[:, m, :], ps[:, j, :], AF.Relu,
                                             bias=bb1[:, m:m + 1])

                emb = actp.tile([128, KE, RB], MD, name=f"emb{blk}", tag="emb")
                for mp in range(2):
                    ps = misc_ps.tile([128, 2, RB], F32, name=f"b2p{blk}_{mp}", tag="ps")
                    for j in range(2):
                        m = mp * 2 + j
                        for k in range(KE):
                            nc.tensor.matmul(ps[:, j, :], w2[:, k, m * 128:(m + 1) * 128],
                                             h1[:, k, :], start=(k == 0), stop=(k == KE - 1))
                    for j in range(2):
                        m = mp * 2 + j
                        nc.scalar.activation(emb[:, m, :], ps[:, j, :], AF.Relu,
                                             bias=bb2[:, m:m + 1])
                embs.append(emb)

                # ci head -> c0, h0 = tanh(c0) = 2*sigmoid(2*c0)-1
                ps = misc_ps.tile([128, 2, RB], F32, name=f"cip{blk}", tag="ps")
                for k in range(KE):
                    nc.tensor.matmul(ps[:, 0, :], ciw[:, k, :], emb[:, k, :],
                                     start=(k == 0), stop=(k == KE - 1))
                c0 = stp.tile([128, RB], F32, name=f"c0_{blk}", tag="c", bufs=6)
                nc.scalar.activation(c0[:], ps[:, 0, :], AF.Identity, bias=cib[:, 0:1])
                sc0 = tmp.tile([128, RB], F32, name=f"sc0_{blk}", tag="tmp")
                nc.scalar.activation(sc0[:], ps[:, 0, :], AF.Sigmoid,
                                     bias=cib[:, 1:2], scale=2.0)
                h0 = stp.tile([128, RB], MD, name=f"h0_{blk}", tag="h", bufs=8)
                nc.gpsimd.tensor_scalar(h0[:], sc0[:], 2.0, -1.0, ALU.mult, ALU.add)
                c0s.append(c0)
                h0s.append(h0)

            for blk in range(NBLK):
                sl = slice(blk * RB, (blk + 1) * RB)
                emb = embs[blk]
                # pol head (rows-major)
                pp = misc_ps.tile([128, 4, A], F32, name=f"polps{blk}", tag="ps")
                for rt in range(4):
                    for k in range(KE):
                        nc.tensor.matmul(pp[:, rt, :], emb[:, k, rt * 128:(rt + 1) * 128],
                                         polw[:, k, :], start=(k == 0), stop=(k == KE - 1))
                lstage = sst.tile([128, 4, A], F32, name=f"lgs{blk}", tag="sstage")
                evac(lstage[:], pp[:])
                nc.sync.dma_start(
                    lg_d[sl, :].rearrange("(rt p) f -> p rt f", p=128), lstage[:])

                # y head (feature-major)
                ystage = ost.tile([128, 5, RB], F32, name=f"ys{blk}", tag="ystage",
                                  bufs=2)
                for mp, mws in ((0, (128, 128)), (1, (128, 128)), (2, (88,))):
                    ps = misc_ps.tile([128, 2, RB], F32, name=f"yp{blk}_{mp}", tag="ps")
                    for j, mw in enumerate(mws):
                        off = sum(YM[:mp * 2 + j])
                        for k in range(KE):
                            nc.tensor.matmul(ps[:mw, j, :], yw[:, k, off:off + mw],
                                             emb[:, k, :], start=(k == 0),
                                             stop=(k == KE - 1))
                    if len(mws) == 2:
                        evac(ystage[:, mp * 2:mp * 2 + 2, :], ps[:])
                    else:
                        evac(ystage[:88, 4, :], ps[:88, 0, :])
                nc.sync.dma_start(
                    yt_d[0:512, sl].rearrange("(m p) r -> p m r", p=128),
                    ystage[:, 0:4, :])
                nc.sync.dma_start(yt_d[512:P, sl], ystage[0:88, 4, :])



            # ---- LSTM scan + per-step heads
            # whh columns are ordered i,f,g,o (as torch); g columns are scaled
            # by 2 on the host so sigmoid with scale=2 computes the tanh parts.
            Cs, Hs = c0s, h0s
            GOFF = (0, 128, 256, 384)
            for a in range(A):
                newC, newH = [None, None], [None, None]
                for blk in range(NBLK):
                    Hp, Cp = Hs[blk], Cs[blk]
                    sg4 = []
                    for j in range(4):     # i, f, g(doubled), o
                        gp = g_ps.tile([128, RB], F32, name=f"g{a}_{blk}_{j}", tag="gps")
                        nc.tensor.matmul(gp[:], whh[:, GOFF[j]:GOFF[j] + 128], Hp[:],
                                         start=True, stop=True)
                        s = tmp.tile([128, RB], F32, name=f"s{a}_{blk}_{j}", tag="tmp")
                        nc.scalar.activation(s[:], gp[:], AF.Sigmoid,
                                             bias=gb[:, a, j:j + 1])
                        sg4.append(s)
                    si, sf, sg, so = sg4
                    # c' = sf*c + si*(2*sg-1) = sf*c + (2*(si*sg) - si)
                    p = tmp.tile([128, RB], F32, name=f"p{a}_{blk}", tag="tmp")
                    nc.gpsimd.tensor_mul(p[:], si[:], sg[:])
                    m1 = tmp.tile([128, RB], F32, name=f"m1_{a}_{blk}", tag="tmp")
                    nc.gpsimd.tensor_mul(m1[:], sf[:], Cp[:])
                    m2 = tmp.tile([128, RB], F32, name=f"m2_{a}_{blk}", tag="tmp")
                    nc.vector.scalar_tensor_tensor(m2[:], p[:], 2.0, si[:],
                                                   ALU.mult, ALU.subtract)
                    Cn = stp.tile([128, RB], F32, name=f"c{a}_{blk}", tag="c", bufs=6)
                    nc.gpsimd.tensor_add(Cn[:], m1[:], m2[:])
                    sc = tmp.tile([128, RB], F32, name=f"sc{a}_{blk}", tag="tmp")
                    nc.scalar.activation(sc[:], Cn[:], AF.Sigmoid, scale=2.0)
                    # h = so*(2*sc-1) = 2*(so*sc) - so
                    w_ = tmp.tile([128, RB], F32, name=f"w{a}_{blk}", tag="tmp")
                    nc.gpsimd.tensor_mul(w_[:], so[:], sc[:])
                    Hn = stp.tile([128, RB], MD, name=f"h{a}_{blk}", tag="h", bufs=8)
                    nc.vector.scalar_tensor_tensor(Hn[:], w_[:], 2.0, so[:],
                                                   ALU.mult, ALU.subtract)
                    newC[blk], newH[blk] = Cn, Hn

                for blk in range(NBLK):
                    sl = slice(blk * RB, (blk + 1) * RB)
                    Hn = newH[blk]
                    # head hidden layers: z,q fused pair + a
                    ups = misc_ps.tile([128, 2, RB], F32, name=f"up{a}_{blk}", tag="ps")
                    upa = misc_ps.tile([128, 2, RB], F32, name=f"ua{a}_{blk}", tag="ps")
                    for j, w in enumerate((zw1, qw1)):
                        nc.tensor.matmul(ups[:, j, :], w[:], Hn[:],
                                         start=True, stop=not u_bias_nonzero)
                        if u_bias_nonzero:
                            nc.tensor.matmul(ups[:, j, :], ub[0:1, j, :], ones[:],
                                             start=False, stop=True)
                    nc.tensor.matmul(upa[:, 0, :], aw1[:], Hn[:],
                                     start=True, stop=not u_bias_nonzero)
                    if u_bias_nonzero:
                        nc.tensor.matmul(upa[:, 0, :], ub[0:1, 2, :], ones[:],
                                         start=False, stop=True)
                    u = upool.tile([128, 2, RB], MD, name=f"u{a}_{blk}", tag="u")
                    nc.vector.tensor_scalar_max(u[:], ups[:], 0.0)
                    ua = upool.tile([128, RB], MD, name=f"ua{a}_{blk}", tag="ua")
                    nc.vector.tensor_scalar_max(ua[:], upa[:, 0, :], 0.0)

                    # z / q output layers (feature-major)
                    for nm, w2t, widths, out_d, uin in (
                        ("z", zw2, YM, zt_d, u[:, 0, :]),
                        ("q", qw2, QM, qt_d, u[:, 1, :]),
                    ):
                        stg = ost.tile([128, 5, RB], OD, name=f"{nm}s{a}_{blk}",
                                       tag="ostage")
                        for mp, mws in ((0, (128, 128)), (1, (128, 128)),
                                        (2, (widths[4],))):
                            ps = misc_ps.tile([128, 2, RB], F32,
                                              name=f"{nm}p{a}_{blk}_{mp}", tag="ps")
                            for j, mw in enumerate(mws):
                                off = sum(widths[:mp * 2 + j])
                                nc.tensor.matmul(ps[:mw, j, :], w2t[:, off:off + mw],
                                                 uin, start=True, stop=True)
                            if len(mws) == 2:
                                evac(stg[:, mp * 2:mp * 2 + 2, :], ps[:])
                            else:
                                evac(stg[:widths[4], 4, :], ps[:widths[4], 0, :])
                        nc.sync.dma_start(
                            out_d[a, 0:512, sl].rearrange("(m p) r -> p m r", p=128),
                            stg[:, 0:4, :])
                        nc.sync.dma_start(out_d[a, 512:sum(widths), sl],
                                          stg[0:widths[4], 4, :])

                    # aux head (rows-major)
                    ap = misc_ps.tile([128, 4, A], F32, name=f"aps{a}_{blk}", tag="ps")
                    for rt in range(4):
                        nc.tensor.matmul(ap[:, rt, :], ua[:, rt * 128:(rt + 1) * 128],
                                         aw2[:], start=True, stop=True)
                    astage = sst.tile([128, 4, A], F32, name=f"as{a}_{blk}", tag="sstage")
                    evac(astage[:], ap[:])
                    nc.sync.dma_start(
                        ax_d[a, sl, :].rearrange("(rt p) f -> p rt f", p=128),
                        astage[:])
                Cs, Hs = newC, newH

    nc.compile()
    return nc


def _get_nc(u_bias_nonzero: bool):
    key = ("v13", u_bias_nonzero)
    if key not in _CACHE:
        _CACHE[key] = _build(u_bias_nonzero)
    return _CACHE[key]


def _prep_shared(inp):
    """Host-side packing of the replicated parameters."""
    def kt(w, ko):          # [ko*128, m] -> [128, ko, m]
        m = w.shape[1]
        return np.ascontiguousarray(
            w.reshape(ko, 128, m).transpose(1, 0, 2)).astype(MDnp)

    beta = (inp["w_ih"] + inp["b_ih"] + inp["b_hh"]).astype(np.float32)  # [A, 4H]
    gb = np.empty((128, A, 4), np.float32)
    for j in range(4):
        gb[:, :, j] = beta[:, j * 128:(j + 1) * 128].T
    gb[:, :, 2] *= 2.0                       # g-gate: sigmoid(2x) trick
    whh = inp["w_hh"].astype(np.float32).copy()
    whh[:, 256:384] *= 2.0                   # g-gate columns pre-doubled

    shared = {
        "w1": kt(inp["bb_w1"], KO),
        "w2": kt(inp["bb_w2"], KE),
        "ciw": kt(inp["ci_w"], KE),
        "yw": kt(inp["y_w"], KE),
        "polw": kt(inp["pol_w"], KE),
        "whh": whh.astype(MDnp),
        "zw1": inp["z_w1"].astype(MDnp),
        "qw1": inp["q_w1"].astype(MDnp),
        "aw1": inp["a_w1"].astype(MDnp),
        "zw2": inp["z_w2"].astype(MDnp),
        "qw2": inp["q_w2"].astype(MDnp),
        "aw2": inp["a_w2"].astype(MDnp),
        "gb": gb,
        "bb1": np.ascontiguousarray(
            inp["bb_b1"].reshape(KE, 128).T).astype(np.float32),
        "bb2": np.ascontiguousarray(
            inp["bb_b2"].reshape(KE, 128).T).astype(np.float32),
        "cib": np.stack([inp["ci_b"], 2.0 * inp["ci_b"]], axis=1).astype(np.float32),
        "ub": np.stack([inp["z_b1"], inp["q_b1"], inp["a_b1"]])[None].astype(MDnp),
    }
    u_bias_nonzero = bool(
        np.any(inp["z_b1"]) or np.any(inp["q_b1"]) or np.any(inp["a_b1"]))
    return shared, u_bias_nonzero


def kernel(**inputs):
    inp = {k: np.asarray(v) for k, v in inputs.items()}
    shared, u_bias_nonzero = _prep_shared(inp)
    nc = _get_nc(u_bias_nonzero)

    in_maps = []
    for c in range(N_CORES):
        bs = c * BC
        x = inp["obs_seq"][:, bs:bs + BC, :].reshape(R, OBS)       # [rows, OBS]
        xt = np.ascontiguousarray(
            x.T.reshape(KO, 128, R).transpose(1, 0, 2)).astype(MDnp)
        m = dict(shared)
        m["xt"] = xt
        in_maps.append(m)

    res = run_bass_kernel_spmd(nc, in_maps, list(range(N_CORES)))

    logits = np.empty((T, B, A), np.float32)
    y = np.empty((T, B, P), np.float32)
    z = np.empty((T, B, A, P), np.float32)
    q = np.empty((T, B, A, NB), np.float32)
    ax = np.empty((T, B, A, A), np.float32)
    for c in range(N_CORES):
        bs = c * BC
        r = res.results[c]
        logits[:, bs:bs + BC] = r["lg"].reshape(T, BC, A)
        y[:, bs:bs + BC] = r["yt"].reshape(P, T, BC).transpose(1, 2, 0)
        z[:, bs:bs + BC] = r["zt"].astype(np.float32).reshape(
            A, P, T, BC).transpose(2, 3, 0, 1)
        q[:, bs:bs + BC] = r["qt"].astype(np.float32).reshape(
            A, NB, T, BC).transpose(2, 3, 0, 1)
        ax[:, bs:bs + BC] = r["ax"].reshape(A, T, BC, A).transpose(1, 2, 0, 3)

    # final-layer biases are applied on the host (all-zero for this module,
    # so these adds normally no-op)
    for arr, b in ((logits, inp["pol_b"]), (y, inp["y_b"]), (z, inp["z_b2"]),
                   (q, inp["q_b2"]), (ax, inp["a_b2"])):
        if np.any(b):
            arr += b
    return logits, y, z, q, ax


# revision 22
# speedup vs baseline: 1.0869x; 1.0869x over previous
"""Trainium2 Bass kernel for nn_DiscoMLPAgent (dense_mlp).

Data-parallel over 8 NeuronCores: obs_seq sharded along B (32 batch rows
per core -> 1024 flattened rows per core), parameters replicated.

On-chip layout is feature-major (features on SBUF partitions, rows on the
free dimension) for the backbone, LSTM, and the wide output heads (y, z, q),
and rows-major for the narrow heads (logits, aux_pi).  Matmul operands are
bf16 (fp32 PSUM accumulation); elementwise state stays fp32.

tanh is computed as 2*sigmoid(2x)-1 so every activation (Sigmoid, Relu,
Identity, Copy) lives in one ACT table set ("sigmoid_and_friends") and the
~2.7us table-switch cost is paid once.  Elementwise work is spread across
ACT, DVE and GpSimd; the LSTM cell uses scalar_tensor_tensor to keep the
recurrence chain short:  c' = sf*c + (2*(si*sg) - si),  h = 2*(so*sc) - so.
"""

import numpy as np
import ml_dtypes

import concourse.bass as bass
import concourse.bacc as bacc
import concourse.tile as tile
import concourse.mybir as mybir
from concourse.bass_utils import run_bass_kernel_spmd

T, B, OBS, A, P, NB, H = 32, 256, 1024, 16, 600, 601, 128
N_CORES = 8
BC = B // N_CORES          # batch rows per core
R = T * BC                 # flattened rows per core (1024)
RB = 512                   # row block (matmul moving free dim)
NBLK = R // RB             # 2
KO = OBS // 128            # 8 k-tiles for obs
KE = 512 // 128            # 4 k-tiles for 512-wide features
YM = [128, 128, 128, 128, 88]    # y / z feature m-tile widths (600)
QM = [128, 128, 128, 128, 89]    # q feature m-tile widths (601)

F32 = mybir.dt.float32
MD = mybir.dt.bfloat16
MDnp = ml_dtypes.bfloat16
OUT_BF16 = True            # z/q written to HBM as bf16, upconverted on host
OD = MD if OUT_BF16 else F32
AF = mybir.ActivationFunctionType
ALU = mybir.AluOpType

_CACHE = {}


def _build(u_bias_nonzero: bool):
    nc = bacc.Bacc(None, target_bir_lowering=False)

    # ---- DRAM I/O (per-core). Host pre-arranges everything to [part, free...]
    xt_d = nc.dram_tensor("xt", [128, KO, R], MD, kind="ExternalInput")
    w1_d = nc.dram_tensor("w1", [128, KO, 512], MD, kind="ExternalInput")
    w2_d = nc.dram_tensor("w2", [128, KE, 512], MD, kind="ExternalInput")
    ciw_d = nc.dram_tensor("ciw", [128, KE, 128], MD, kind="ExternalInput")
    yw_d = nc.dram_tensor("yw", [128, KE, P], MD, kind="ExternalInput")
    polw_d = nc.dram_tensor("polw", [128, KE, A], MD, kind="ExternalInput")
    whh_d = nc.dram_tensor("whh", [128, 4 * H], MD, kind="ExternalInput")
    zw1_d = nc.dram_tensor("zw1", [128, H], MD, kind="ExternalInput")
    qw1_d = nc.dram_tensor("qw1", [128, H], MD, kind="ExternalInput")
    aw1_d = nc.dram_tensor("aw1", [128, H], MD, kind="ExternalInput")
    zw2_d = nc.dram_tensor("zw2", [128, P], MD, kind="ExternalInput")
    qw2_d = nc.dram_tensor("qw2", [128, NB], MD, kind="ExternalInput")
    aw2_d = nc.dram_tensor("aw2", [128, A], MD, kind="ExternalInput")
    gb_d = nc.dram_tensor("gb", [128, A, 4], F32, kind="ExternalInput")
    bb1_d = nc.dram_tensor("bb1", [128, KE], F32, kind="ExternalInput")
    bb2_d = nc.dram_tensor("bb2", [128, KE], F32, kind="ExternalInput")
    cib_d = nc.dram_tensor("cib", [128, 2], F32, kind="ExternalInput")
    ub_d = nc.dram_tensor("ub", [1, 3, 128], MD, kind="ExternalInput")

    zt_d = nc.dram_tensor("zt", [A, P, R], OD, kind="ExternalOutput")
    qt_d = nc.dram_tensor("qt", [A, NB, R], OD, kind="ExternalOutput")
    yt_d = nc.dram_tensor("yt", [P, R], F32, kind="ExternalOutput")
    lg_d = nc.dram_tensor("lg", [R, A], F32, kind="ExternalOutput")
    ax_d = nc.dram_tensor("ax", [A, R, A], F32, kind="ExternalOutput")

    # greedy cost-balanced assignment of PSUM->SBUF copies; counters are
    # primed with each engine's fixed (non-copy) load in ns.
    eng_load = {"V": 93000.0, "A": 110000.0}

    def evac(dst, src):
        """PSUM -> SBUF copy, assigned to the less-loaded of DVE/ACT."""
        n = 1
        for d in dst.shape[1:]:
            n *= d
        if eng_load["V"] <= eng_load["A"]:
            eng_load["V"] += (n + 178) / 0.96
            nc.vector.tensor_copy(dst, src)
        else:
            eng_load["A"] += (n + 352) / 1.2
            nc.scalar.activation(dst, src, AF.Copy)

    def relu(dst, src):
        """PSUM -> SBUF relu, assigned to the less-loaded of DVE/ACT."""
        n = 1
        for d in dst.shape[1:]:
            n *= d
        if eng_load["V"] <= eng_load["A"]:
            eng_load["V"] += (n + 178) / 0.96
            nc.vector.tensor_scalar_max(dst, src, 0.0)
        else:
            eng_load["A"] += (n + 352) / 1.2
            nc.scalar.activation(dst, src, AF.Relu)

    with tile.TileContext(nc) as tc:
        with (
            tc.tile_pool(name="const", bufs=1) as cp,
            tc.tile_pool(name="xtp", bufs=2) as xtp,
            tc.tile_pool(name="actp", bufs=2) as actp,
            tc.tile_pool(name="state", bufs=1) as stp,
            tc.tile_pool(name="tmp", bufs=16) as tmp,
            tc.tile_pool(name="up", bufs=3) as upool,
            tc.tile_pool(name="ostage", bufs=6) as ost,
            tc.tile_pool(name="sstage", bufs=6) as sst,
            tc.tile_pool(name="g_ps", bufs=2, space=bass.MemorySpace.PSUM) as g_ps,
            tc.tile_pool(name="misc_ps", bufs=3, space=bass.MemorySpace.PSUM) as misc_ps,
        ):
            # ---- load constants
            def cload(dram, shape, dtype, nm):
                t = cp.tile(shape, dtype, name=nm, tag=nm)
                nc.sync.dma_start(t[:], dram[:])
                return t

            w1 = cload(w1_d, [128, KO, 512], MD, "w1")
            w2 = cload(w2_d, [128, KE, 512], MD, "w2")
            ciw = cload(ciw_d, [128, KE, 128], MD, "ciw")
            yw = cload(yw_d, [128, KE, P], MD, "yw")
            polw = cload(polw_d, [128, KE, A], MD, "polw")
            whh = cload(whh_d, [128, 4 * H], MD, "whh")
            zw1 = cload(zw1_d, [128, H], MD, "zw1")
            qw1 = cload(qw1_d, [128, H], MD, "qw1")
            aw1 = cload(aw1_d, [128, H], MD, "aw1")
            zw2 = cload(zw2_d, [128, P], MD, "zw2")
            qw2 = cload(qw2_d, [128, NB], MD, "qw2")
            aw2 = cload(aw2_d, [128, A], MD, "aw2")
            gb = cload(gb_d, [128, A, 4], F32, "gb")
            bb1 = cload(bb1_d, [128, KE], F32, "bb1")
            bb2 = cload(bb2_d, [128, KE], F32, "bb2")
            cib = cload(cib_d, [128, 2], F32, "cib")
            if u_bias_nonzero:
                ub = cload(ub_d, [1, 3, 128], MD, "ub")
                ones = cp.tile([1, RB], MD, name="ones", tag="ones")
                nc.vector.memset(ones[:], 1.0)

            embs, h0s, c0s = [], [], []

            # ---- backbone + feedforward heads, per row-block
            for blk in range(NBLK):
                sl = slice(blk * RB, (blk + 1) * RB)
                xt = xtp.tile([128, KO, RB], MD, name=f"xt{blk}", tag="xt")
                nc.sync.dma_start(xt[:], xt_d[:, :, sl])

                h1 = actp.tile([128, KE, RB], MD, name=f"h1_{blk}", tag="h1")
                for mp in range(2):
                    ps = misc_ps.tile([128, 2, RB], F32, name=f"b1p{blk}_{mp}", tag="ps")
                    for j in range(2):
                        m = mp * 2 + j
                        for k in range(KO):
                            nc.tensor.matmul(ps[:, j, :], w1[:, k, m * 128:(m + 1) * 128],
                                             xt[:, k, :], start=(k == 0), stop=(k == KO - 1))
                    for j in range(2):
                        m = mp * 2 + j
                        nc.scalar.activation(h1[:, m, :], ps[:, j, :], AF.Relu,
                                             bias=bb1[:, m:m + 1])

                emb = actp.tile([128, KE, RB], MD, name=f"emb{blk}", tag="emb")
                for mp in range(2):
                    ps = misc_ps.tile([128, 2, RB], F32, name=f"b2p{blk}_{mp}", tag="ps")
                    for j in range(2):
                        m = mp * 2 + j
                        for k in range(KE):
                            nc.tensor.matmul(ps[:, j, :], w2[:, k, m * 128:(m + 1) * 128],
                                             h1[:, k, :], start=(k == 0), stop=(k == KE - 1))
                    for j in range(2):
                        m = mp * 2 + j
                        nc.scalar.activation(emb[:, m, :], ps[:, j, :], AF.Relu,
                                             bias=bb2[:, m:m + 1])
                embs.append(emb)

                # ci head -> c0, h0 = tanh(c0) = 2*sigmoid(2*c0)-1
                ps = misc_ps.tile([128, 2, RB], F32, name=f"cip{blk}", tag="ps")
                for k in range(KE):
                    nc.tensor.matmul(ps[:, 0, :], ciw[:, k, :], emb[:, k, :],
                                     start=(k == 0), stop=(k == KE - 1))
                c0 = stp.tile([128, RB], F32, name=f"c0_{blk}", tag="c", bufs=6)
                nc.scalar.activation(c0[:], ps[:, 0, :], AF.Identity, bias=cib[:, 0:1])
                sc0 = tmp.tile([128, RB], F32, name=f"sc0_{blk}", tag="tmp")
                nc.scalar.activation(sc0[:], ps[:, 0, :], AF.Sigmoid,
                                     bias=cib[:, 1:2], scale=2.0)
                h0 = stp.tile([128, RB], MD, name=f"h0_{blk}", tag="h", bufs=8)
                nc.gpsimd.tensor_scalar(h0[:], sc0[:], 2.0, -1.0, ALU.mult, ALU.add)
                c0s.append(c0)
                h0s.append(h0)

            for blk in range(NBLK):
                sl = slice(blk * RB, (blk + 1) * RB)
                emb = embs[blk]
                # pol head (rows-major)
                pp = misc_ps.tile([128, 4, A], F32, name=f"polps{blk}", tag="ps")
                for rt in range(4):
                    for k in range(KE):
                        nc.tensor.matmul(pp[:, rt, :], emb[:, k, rt * 128:(rt + 1) * 128],
                                         polw[:, k, :], start=(k == 0), stop=(k == KE - 1))
                lstage = sst.tile([128, 4, A], F32, name=f"lgs{blk}", tag="sstage")
                evac(lstage[:], pp[:])
                nc.sync.dma_start(
                    lg_d[sl, :].rearrange("(rt p) f -> p rt f", p=128), lstage[:])

                # y head (feature-major)
                ystage = ost.tile([128, 5, RB], F32, name=f"ys{blk}", tag="ystage",
                                  bufs=2)
                for mp, mws in ((0, (128, 128)), (1, (128, 128)), (2, (88,))):
                    ps = misc_ps.tile([128, 2, RB], F32, name=f"yp{blk}_{mp}", tag="ps")
                    for j, mw in enumerate(mws):
                        off = sum(YM[:mp * 2 + j])
                        for k in range(KE):
                            nc.tensor.matmul(ps[:mw, j, :], yw[:, k, off:off + mw],
                                             emb[:, k, :], start=(k == 0),
                                             stop=(k == KE - 1))
                    if len(mws) == 2:
                        evac(ystage[:, mp * 2:mp * 2 + 2, :], ps[:])
                    else:
                        evac(ystage[:88, 4, :], ps[:88, 0, :])
                nc.sync.dma_start(
                    yt_d[0:512, sl].rearrange("(m p) r -> p m r", p=128),
                    ystage[:, 0:4, :])
                nc.sync.dma_start(yt_d[512:P, sl], ystage[0:88, 4, :])



            # ---- LSTM scan + per-step heads
            # whh columns are ordered i,f,g,o (as torch); g columns are scaled
            # by 2 on the host so sigmoid with scale=2 computes the tanh parts.
            Cs, Hs = c0s, h0s
            GOFF = (0, 128, 256, 384)
            for a in range(A):
                newC, newH = [None, None], [None, None]
                for blk in range(NBLK):
                    Hp, Cp = Hs[blk], Cs[blk]
                    sg4 = []
                    for j in range(4):     # i, f, g(doubled), o
                        gp = g_ps.tile([128, RB], F32, name=f"g{a}_{blk}_{j}", tag="gps")
                        nc.tensor.matmul(gp[:], whh[:, GOFF[j]:GOFF[j] + 128], Hp[:],
                                         start=True, stop=True)
                        s = tmp.tile([128, RB], F32, name=f"s{a}_{blk}_{j}", tag="tmp")
                        nc.scalar.activation(s[:], gp[:], AF.Sigmoid,
                                             bias=gb[:, a, j:j + 1])
                        sg4.append(s)
                    si, sf, sg, so = sg4
                    # c' = sf*c + si*tg,  tg = 2*sg-1 (tanh via sigmoid)
                    tg = tmp.tile([128, RB], F32, name=f"tg{a}_{blk}", tag="tmp")
                    nc.gpsimd.tensor_scalar(tg[:], sg[:], 2.0, -1.0, ALU.mult, ALU.add)
                    m1 = tmp.tile([128, RB], F32, name=f"m1_{a}_{blk}", tag="tmp")
                    nc.gpsimd.tensor_mul(m1[:], sf[:], Cp[:])
                    m2 = tmp.tile([128, RB], F32, name=f"m2_{a}_{blk}", tag="tmp")
                    nc.gpsimd.tensor_mul(m2[:], si[:], tg[:])
                    Cn = stp.tile([128, RB], F32, name=f"c{a}_{blk}", tag="c", bufs=6)
                    nc.gpsimd.tensor_add(Cn[:], m1[:], m2[:])
                    sc = tmp.tile([128, RB], F32, name=f"sc{a}_{blk}", tag="tmp")
                    nc.scalar.activation(sc[:], Cn[:], AF.Sigmoid, scale=2.0)
                    # h = so*(2*sc-1) = 2*(so*sc) - so   (STT keeps DVE chain short)
                    w_ = tmp.tile([128, RB], F32, name=f"w{a}_{blk}", tag="tmp")
                    nc.gpsimd.tensor_mul(w_[:], so[:], sc[:])
                    Hn = stp.tile([128, RB], MD, name=f"h{a}_{blk}", tag="h", bufs=8)
                    nc.vector.scalar_tensor_tensor(Hn[:], w_[:], 2.0, so[:],
                                                   ALU.mult, ALU.subtract)
                    newC[blk], newH[blk] = Cn, Hn

                for blk in range(NBLK):
                    sl = slice(blk * RB, (blk + 1) * RB)
                    Hn = newH[blk]
                    # head hidden layers: z,q fused pair + a
                    ups = misc_ps.tile([128, 2, RB], F32, name=f"up{a}_{blk}", tag="ps")
                    upa = misc_ps.tile([128, 2, RB], F32, name=f"ua{a}_{blk}", tag="ps")
                    for j, w in enumerate((zw1, qw1)):
                        nc.tensor.matmul(ups[:, j, :], w[:], Hn[:],
                                         start=True, stop=not u_bias_nonzero)
                        if u_bias_nonzero:
                            nc.tensor.matmul(ups[:, j, :], ub[0:1, j, :], ones[:],
                                             start=False, stop=True)
                    nc.tensor.matmul(upa[:, 0, :], aw1[:], Hn[:],
                                     start=True, stop=not u_bias_nonzero)
                    if u_bias_nonzero:
                        nc.tensor.matmul(upa[:, 0, :], ub[0:1, 2, :], ones[:],
                                         start=False, stop=True)
                    u = upool.tile([128, 2, RB], MD, name=f"u{a}_{blk}", tag="u")
                    nc.vector.tensor_scalar_max(u[:], ups[:], 0.0)
                    ua = upool.tile([128, RB], MD, name=f"ua{a}_{blk}", tag="ua")
                    nc.vector.tensor_scalar_max(ua[:], upa[:, 0, :], 0.0)

                    # z / q output layers (feature-major)
                    for nm, w2t, widths, out_d, uin in (
                        ("z", zw2, YM, zt_d, u[:, 0, :]),
                        ("q", qw2, QM, qt_d, u[:, 1, :]),
                    ):
                        stg = ost.tile([128, 5, RB], OD, name=f"{nm}s{a}_{blk}",
                                       tag="ostage")
                        for mp, mws in ((0, (128, 128)), (1, (128, 128)),
                                        (2, (widths[4],))):
                            ps = misc_ps.tile([128, 2, RB], F32,
                                              name=f"{nm}p{a}_{blk}_{mp}", tag="ps")
                            for j, mw in enumerate(mws):
                                off = sum(widths[:mp * 2 + j])
                                nc.tensor.matmul(ps[:mw, j, :], w2t[:, off:off + mw],
                                                 uin, start=True, stop=True)
                            if len(mws) == 2:
                                evac(stg[:, mp * 2:mp * 2 + 2, :], ps[:])
                            else:
                                evac(stg[:widths[4], 4, :], ps[:widths[4], 0, :])
                        nc.sync.dma_start(
                            out_d[a, 0:512, sl].rearrange("(m p) r -> p m r", p=128),
                            stg[:, 0:4, :])
                        nc.sync.dma_start(out_d[a, 512:sum(widths), sl],
                                          stg[0:widths[4], 4, :])

                    # aux head (rows-major)
                    ap = misc_ps.tile([128, 4, A], F32, name=f"aps{a}_{blk}", tag="ps")
                    for rt in range(4):
                        nc.tensor.matmul(ap[:, rt, :], ua[:, rt * 128:(rt + 1) * 128],
                                         aw2[:], start=True, stop=True)
                    astage = sst.tile([128, 4, A], F32, name=f"as{a}_{blk}", tag="sstage")
                    evac(astage[:], ap[:])
                    nc.sync.dma_start(
                        ax_d[a, sl, :].rearrange("(rt p) f -> p rt f", p=128),
                        astage[:])
                Cs, Hs = newC, newH

    nc.compile()
    return nc


def _get_nc(u_bias_nonzero: bool):
    key = ("v17", u_bias_nonzero)
    if key not in _CACHE:
        _CACHE[key] = _build(u_bias_nonzero)
    return _CACHE[key]


def _prep_shared(inp):
    """Host-side packing of the replicated parameters."""
    def kt(w, ko):          # [ko*128, m] -> [128, ko, m]
        m = w.shape[1]
        return np.ascontiguousarray(
            w.reshape(ko, 128, m).transpose(1, 0, 2)).astype(MDnp)

    beta = (inp["w_ih"] + inp["b_ih"] + inp["b_hh"]).astype(np.float32)  # [A, 4H]
    gb = np.empty((128, A, 4), np.float32)
    for j in range(4):
        gb[:, :, j] = beta[:, j * 128:(j + 1) * 128].T
    gb[:, :, 2] *= 2.0                       # g-gate: sigmoid(2x) trick
    whh = inp["w_hh"].astype(np.float32).copy()
    whh[:, 256:384] *= 2.0                   # g-gate columns pre-doubled

    shared = {
        "w1": kt(inp["bb_w1"], KO),
        "w2": kt(inp["bb_w2"], KE),
        "ciw": kt(inp["ci_w"], KE),
        "yw": kt(inp["y_w"], KE),
        "polw": kt(inp["pol_w"], KE),
        "whh": whh.astype(MDnp),
        "zw1": inp["z_w1"].astype(MDnp),
        "qw1": inp["q_w1"].astype(MDnp),
        "aw1": inp["a_w1"].astype(MDnp),
        "zw2": inp["z_w2"].astype(MDnp),
        "qw2": inp["q_w2"].astype(MDnp),
        "aw2": inp["a_w2"].astype(MDnp),
        "gb": gb,
        "bb1": np.ascontiguousarray(
            inp["bb_b1"].reshape(KE, 128).T).astype(np.float32),
        "bb2": np.ascontiguousarray(
            inp["bb_b2"].reshape(KE, 128).T).astype(np.float32),
        "cib": np.stack([inp["ci_b"], 2.0 * inp["ci_b"]], axis=1).astype(np.float32),
        "ub": np.stack([inp["z_b1"], inp["q_b1"], inp["a_b1"]])[None].astype(MDnp),
    }
    u_bias_nonzero = bool(
        np.any(inp["z_b1"]) or np.any(inp["q_b1"]) or np.any(inp["a_b1"]))
    return shared, u_bias_nonzero


def kernel(**inputs):
    inp = {k: np.asarray(v) for k, v in inputs.items()}
    shared, u_bias_nonzero = _prep_shared(inp)
    nc = _get_nc(u_bias_nonzero)

    in_maps = []
    for c in range(N_CORES):
        bs = c * BC
        x = inp["obs_seq"][:, bs:bs + BC, :].reshape(R, OBS)       # [rows, OBS]
        xt = np.ascontiguousarray(
            x.T.reshape(KO, 128, R).transpose(1, 0, 2)).astype(MDnp)
        m = dict(shared)
        m["xt"] = xt
        in_maps.append(m)

    res = run_bass_kernel_spmd(nc, in_maps, list(range(N_CORES)))

    logits = np.empty((T, B, A), np.float32)
    y = np.empty((T, B, P), np.float32)
    z = np.empty((T, B, A, P), np.float32)
    q = np.empty((T, B, A, NB), np.float32)
    ax = np.empty((T, B, A, A), np.float32)
    for c in range(N_CORES):
        bs = c * BC
        r = res.results[c]
        logits[:, bs:bs + BC] = r["lg"].reshape(T, BC, A)
        y[:, bs:bs + BC] = r["yt"].reshape(P, T, BC).transpose(1, 2, 0)
        z[:, bs:bs + BC] = r["zt"].astype(np.float32).reshape(
            A, P, T, BC).transpose(2, 3, 0, 1)
        q[:, bs:bs + BC] = r["qt"].astype(np.float32).reshape(
            A, NB, T, BC).transpose(2, 3, 0, 1)
        ax[:, bs:bs + BC] = r["ax"].reshape(A, T, BC, A).transpose(1, 2, 0, 3)

    # final-layer biases are applied on the host (all-zero for this module,
    # so these adds normally no-op)
    for arr, b in ((logits, inp["pol_b"]), (y, inp["y_b"]), (z, inp["z_b2"]),
                   (q, inp["q_b2"]), (ax, inp["a_b2"])):
        if np.any(b):
            arr += b
    return logits, y, z, q, ax
